# revision 57
# baseline (speedup 1.0000x reference)
"""Bass/Tile TRN2 kernel for nn_MultiHeadAttention_74019466380032.

Reference computation (C=6 combs, H=8 heads, U=100, D=512, T=512, B=16):
    q/k/v = einsum('btd,chdu->chbtu', x, W{Q,K,V})
    qw    = einsum('chbtu,chuv->chbtv', q, Wdot)
    logits= einsum('chbtv,chbsv->chbts', qw, k);  attn = softmax(logits, -1)
    o     = einsum('chbts,chbsu->chbtu', attn, v)
    z     = einsum('cbto,cou->cbtu', concat_heads(o), WO) + b;  leaky_relu(z, 0.1)

Sharding: data-parallel over batch B — each of the 8 cores computes all 48
(c,h) head attentions for its 2 batches.  No cross-core communication.

Device layout: "transposed" activations (feature on partitions, tokens on the
free axis) so every contraction dim lands on partitions:
    qwT,kT (U=100p, 1024t)   lhsT=(WQ@Wdot | WK) (d,u) stationary, rhs=xT (d,t)
    logitsT(s=128p, t=512)   lhsT=kT s-chunk (u,s), rhs=qwT b-slice (u,t)
    PT     = exp(logitsT)    (scalar engine; logits bounded ~ +-40, no max pass)
    oT     (101p, 512t)      lhsT=v_aug s-tile (s, U+1): v cols are
                             [v0..95, ones, v96..99] so the softmax Z
                             (= sum_s P) accumulates into PSUM partition 96,
                             a legal engine base partition
    zT     (100p, 512t)      lhsT=WO head block (101,u') accumulated over the
                             8 heads (WO row 96 is zero: kills the Z row)
    out    PE-transpose zT back to (t,u), +bias, leaky-relu, DMA out.
Softmax 1/Z (free-axis scale) is applied at the oT PSUM->SBUF copy:
DVE reciprocal of the Z row -> gpsimd partition_broadcast -> tensor_mul.
x and projection weights travel in fp16 (not bf16): x enters the logits
quadratically and exp() amplifies the correlated rounding; fp16 runs at the
same PE rate and cuts the end-to-end error ~8x (1.1e-2 -> 1.4e-3).
"""

import os
import sys

for _p in ("/opt/trn_rl_repo", "/root/.axon_site/_ro/trn_rl_repo"):
    if os.path.isdir(_p) and _p not in sys.path:
        sys.path.insert(0, _p)

import numpy as np
import ml_dtypes

import concourse.bass as bass
import concourse.tile as tile
import concourse.mybir as mybir
from concourse import bacc
from concourse.bass_utils import run_bass_kernel_spmd
from concourse.masks import make_identity

F32 = mybir.dt.float32
BF16 = mybir.dt.bfloat16
FP16 = mybir.dt.float16
AF = mybir.ActivationFunctionType

C, H, U, D, T, B = 6, 8, 100, 512, 512, 16
N_CORES = 8
BC = B // N_CORES          # batches per core
TOK = BC * T               # tokens per core (1024)
NH = C * H                 # 48 head attentions
KT = D // 128              # 4 k-tiles over feature dim
ST = T // 128              # 4 s-tiles per batch
TT = TOK // 128            # 8 token tiles per core
UA = U + 1                 # v augmented with a ones column (Z row)
ZP = 96                    # partition where the Z row lands (legal engine base)
# v_aug column order per head: [v0..v95, ones, v96..v99] -> Z at partition 96

# logits matmul precision: bf16 rounding of qwT/kT is amplified by exp() on
# large-|logit| rows; f32/f32r inputs fix it at 2.0x/1.5x PE cost for logits.
LOGITS_DT = os.environ.get("LOGITS_DT", "bf16")


def build_kernel(nc, rep=1):
    """Emit the Tile program. rep>1 repeats the whole computation (timing)."""
    xT_d = nc.dram_tensor("xt", [128, KT, TOK], FP16, kind="ExternalInput")
    # wq is pre-multiplied by Wdot on the host (q only ever feeds the
    # bilinear transform): qwT = (WQ @ Wdot)^T-projection of x.
    wq_d = nc.dram_tensor("wq", [C, H, 128, KT, U], FP16, kind="ExternalInput")
    wk_d = nc.dram_tensor("wk", [C, H, 128, KT, U], FP16, kind="ExternalInput")
    wv_d = nc.dram_tensor("wv", [2, 128, KT, NH * UA // 2], FP16, kind="ExternalInput")
    wo_d = nc.dram_tensor("wo", [C, UA, H, U], BF16, kind="ExternalInput")
    bt_d = nc.dram_tensor("bt", [U, T], F32, kind="ExternalInput")
    out_d = nc.dram_tensor("out", [C, BC, T, U], F32, kind="ExternalOutput")

    from contextlib import ExitStack

    with tile.TileContext(nc) as tc, ExitStack() as ctx:
        ep = ctx.enter_context
        constp = ep(tc.tile_pool(name="const", bufs=1))
        wvp = ep(tc.tile_pool(name="wvp", bufs=2))
        vallp = ep(tc.tile_pool(name="vall", bufs=1))
        wts = ep(tc.tile_pool(name="wts", bufs=4))
        wop = ep(tc.tile_pool(name="wop", bufs=2))
        qkp = ep(tc.tile_pool(name="qk", bufs=4))
        qwp = ep(tc.tile_pool(name="qw", bufs=4))
        ptp = ep(tc.tile_pool(name="pt", bufs=12))
        znp = ep(tc.tile_pool(name="zn", bufs=6))
        otp = ep(tc.tile_pool(name="ot", bufs=6))
        zfp = ep(tc.tile_pool(name="zf", bufs=2))
        zop = ep(tc.tile_pool(name="zo", bufs=4))
        psA = ep(tc.tile_pool(name="psA", bufs=2, space="PSUM"))
        psL = ep(tc.tile_pool(name="psL", bufs=2, space="PSUM"))
        psO = ep(tc.tile_pool(name="psO", bufs=2, space="PSUM"))
        psW = ep(tc.tile_pool(name="psW", bufs=2, space="PSUM"))
        if True:
            # ---- constants ----
            xT = constp.tile([128, KT, TOK], FP16)
            for kt in range(KT):
                nc.sync.dma_start(xT[:, kt, :], xT_d[:, kt, :])
            ident = constp.tile([128, 128], F32)
            make_identity(nc, ident[:])
            zbias = constp.tile([128, 1], F32)
            nc.vector.memset(zbias[:], 0.0)
            bT = None

            for _r in range(rep):
                # ---- v for all 48 heads, natural layout, 101 cols per head.
                # Emitted lazily (interleaved with the main loop) so the psL
                # pool's slot rotation doesn't serialize the first logits
                # behind the whole v sweep. ----
                v_all = vallp.tile([128, TT, NH, UA], BF16, tag="vall")
                # ones column (Z accumulator) at index ZP=96 of every head,
                # set FIRST so it is not a barrier behind all v copies
                nc.vector.memset(v_all[:, :, :, ZP : ZP + 1], 1.0)
                half_w = NH * UA // 2  # 2424 columns per half (24 heads)
                # 5-head chunks (505 cols <= one psum bank): 10 chunks total
                vchunks = []
                for half in range(2):
                    for ci in range(5):
                        c0 = ci * 5 * UA if ci < 4 else 20 * UA
                        cw = 5 * UA if ci < 4 else 4 * UA
                        vchunks.append((half, c0, cw))
                wv_tiles = {}
                vstate = {"emitted": 0}

                def emit_v_chunk():
                    half, c0, cw = vchunks[vstate["emitted"]]
                    vstate["emitted"] += 1
                    if half not in wv_tiles:
                        wv_sb = wvp.tile(
                            [128, KT, half_w], FP16, tag="wv", name=f"wv{half}"
                        )
                        wv_tiles[half] = wv_sb
                    wv_sb = wv_tiles[half]
                    # per-chunk DMA slices: range-based deps let this chunk's
                    # matmuls start ~2.6us after issue instead of waiting the
                    # full 2.4MB half transfer
                    for kt in range(KT):
                        nc.sync.dma_start(
                            wv_sb[:, kt, c0 : c0 + cw], wv_d[half, :, kt, c0 : c0 + cw]
                        )
                    h0 = (half * half_w + c0) // UA
                    nhc = cw // UA
                    for s in range(TT):
                        pv = psL.tile([128, cw], F32, tag="lg", name=f"pv{h0}_{s}")
                        for kt in range(KT):
                            nc.tensor.matmul(
                                pv[:],
                                xT[:, kt, s * 128 : (s + 1) * 128],
                                wv_sb[:, kt, c0 : c0 + cw],
                                start=(kt == 0),
                                stop=(kt == KT - 1),
                            )
                        pv_h = pv[:].rearrange("p (h u) -> p h u", u=UA)
                        # skip the ones column (ZP): keep the memset intact
                        nc.vector.tensor_copy(
                            v_all[:, s, h0 : h0 + nhc, 0:ZP], pv_h[:, :, 0:ZP]
                        )
                        nc.vector.tensor_copy(
                            v_all[:, s, h0 : h0 + nhc, ZP + 1 : UA],
                            pv_h[:, :, ZP + 1 : UA],
                        )

                def ensure_v(ch):
                    # chunks cover 5 heads each; stay one chunk ahead
                    want = min(ch // 5 + 2, len(vchunks))
                    while vstate["emitted"] < want:
                        emit_v_chunk()

                ensure_v(0)
                lg_dt = BF16 if LOGITS_DT == "bf16" else F32
                state = {"bT": None}

                def emit_head(c, h):
                    """Weight DMAs + qwT/kT projections for head (c,h)."""
                    wq_t = wts.tile([128, KT, U], FP16, tag="wq", name=f"wq{c}_{h}")
                    nc.sync.dma_start(wq_t[:], wq_d[c, h])
                    wk_t = wts.tile([128, KT, U], FP16, tag="wk", name=f"wk{c}_{h}")
                    nc.sync.dma_start(wk_t[:], wk_d[c, h])
                    ensure_v(c * H + h)
                    qwT = qwp.tile([U, TOK], lg_dt, tag="qwT", name=f"qwT{c}_{h}")
                    kT = qkp.tile([U, TOK], lg_dt, tag="kT", name=f"kT{c}_{h}")
                    for w_t, dst, eng in ((wq_t, qwT, "v"), (wk_t, kT, "s")):
                        for hf in range(2):
                            pp = psA.tile([U, 512], F32, tag="proj", name="pp")
                            for kt in range(KT):
                                nc.tensor.matmul(
                                    pp[:],
                                    w_t[:, kt, :],
                                    xT[:, kt, hf * 512 : (hf + 1) * 512],
                                    start=(kt == 0),
                                    stop=(kt == KT - 1),
                                )
                            nc.vector.tensor_copy(
                                dst[:, hf * 512 : (hf + 1) * 512], pp[:]
                            )
                    return qwT, kT

                def emit_attn(c, h, qwT, kT, wo_t, pzw, fin=False):
                    """logits -> exp -> oT -> normalize -> WO contribution.

                    Both batches' logits are emitted before either batch's
                    o-matmuls so the PE never chases its own exp() latency.
                    fin=True (last comb only): finalize each batch as soon as
                    its WO accumulation completes, to shorten the kernel tail.
                    """
                    ch = c * H + h
                    for b in range(BC):
                        pts = []
                        for s in range(ST):
                            pl = psL.tile([128, T], F32, tag="lg", name="pl")
                            lhs_ap = kT[:, b * T + s * 128 : b * T + (s + 1) * 128]
                            rhs_ap = qwT[:, b * T : (b + 1) * T]
                            if LOGITS_DT == "f32r":
                                lhs_ap = lhs_ap.bitcast(mybir.dt.float32r)
                                rhs_ap = rhs_ap.bitcast(mybir.dt.float32r)
                            nc.tensor.matmul(
                                pl[:], lhs_ap, rhs_ap, start=True, stop=True
                            )
                            pt = ptp.tile([128, T], BF16, tag="pt", name="pt")
                            nc.scalar.activation(pt[:], pl[:], AF.Exp, bias=zbias[:])
                            pts.append(pt)
                        po = psO.tile([UA, T], F32, tag="ot", name="po")
                        for s in range(ST):
                            nc.tensor.matmul(
                                po[:],
                                v_all[:, b * ST + s, ch, :],
                                pts[s][:],
                                start=(s == 0),
                                stop=(s == ST - 1),
                            )
                        zinv = znp.tile([1, T], F32, tag="zinv", name="zinv")
                        nc.vector.reciprocal(zinv[:], po[ZP : ZP + 1, :])
                        zbc = znp.tile([UA, T], F32, tag="zbc", name="zbc")
                        nc.gpsimd.partition_broadcast(zbc[:], zinv[:])
                        on = otp.tile([UA, T], BF16, tag="on", name="on")
                        # row 96 (Z * Zinv = 1) is killed by the WO zero row
                        nc.vector.tensor_mul(on[:], po[:], zbc[:])
                        nc.tensor.matmul(
                            pzw[b][:],
                            wo_t[:, h, :],
                            on[:],
                            start=(h == 0),
                            stop=(h == H - 1),
                        )
                        if fin and h == H - 1:
                            emit_bias_fin_b(c, b, pzw)

                def emit_bias(c, pzw):
                    """+bias (frees the psW banks), leaky relu — all emitted
                    two heads before the transposes consume za, so the ACT/DVE
                    queues drain long before the PE reaches the transposes."""
                    bT = state["bT"]
                    zas = []
                    for b in range(BC):
                        zb = zfp.tile([U, T], F32, tag="zb", name="zb")
                        nc.vector.tensor_add(zb[:], pzw[b][:], bT[:])
                        zs = zfp.tile([U, T], F32, tag="zs", name="zs")
                        nc.scalar.mul(zs[:], zb[:], 0.1)
                        za = zfp.tile([U, T], F32, tag="za", name="za")
                        nc.vector.tensor_max(za[:], zb[:], zs[:])
                        zas.append(za)
                    return zas

                def emit_finalize_b(c, b, za):
                    """PE transpose back to (t,u), DMA out."""
                    for tt in range(ST):
                        pzt = psO.tile([128, U], F32, tag="ot", name="pzt")
                        nc.tensor.transpose(
                            pzt[:],
                            za[:, tt * 128 : (tt + 1) * 128],
                            ident[0:U, 0:U],
                        )
                        zo = zop.tile([128, U], F32, tag="zo", name="zo")
                        nc.vector.tensor_copy(zo[:], pzt[:])
                        nc.gpsimd.dma_start(
                            out_d[c, b, tt * 128 : (tt + 1) * 128, :], zo[:]
                        )

                def emit_finalize(c, zbs):
                    for b in range(BC):
                        emit_finalize_b(c, b, zbs[b])

                def emit_bias_fin_b(c, b, pzw):
                    bT = state["bT"]
                    zb = zfp.tile([U, T], F32, tag="zb", name="zb")
                    nc.vector.tensor_add(zb[:], pzw[b][:], bT[:])
                    emit_finalize_b(c, b, zb)

                # ---- main loop, software-pipelined one head deep: head h+1's
                # projections are emitted before head h's attention so the PE
                # stream never head-of-line blocks on fresh qwT/kT copies, and
                # comb finalization is deferred into the next comb's stream.
                pending_attn = None   # (c, h, qwT, kT, wo_t, pzw)
                pending_fin = None    # (c, zbs)
                for c in range(C):
                    wo_t = wop.tile([UA, H, U], BF16, tag="wo", name=f"wo{c}")
                    pzw = [
                        psW.tile([U, T], F32, tag="wz", name=f"pzw{c}_{bb}")
                        for bb in range(BC)
                    ]
                    for h in range(H):
                        qwT, kT = emit_head(c, h)
                        if h == 0:
                            nc.sync.dma_start(wo_t[:], wo_d[c])
                        if c == 0 and h == 3:
                            state["bT"] = constp.tile([U, T], F32, name="bT")
                            nc.sync.dma_start(state["bT"][:], bt_d.ap())
                        if pending_attn is not None:
                            pa = pending_attn
                            emit_attn(*pa)
                            if pa[1] == H - 1:
                                # last head of comb pa[0]: free its psW banks
                                pending_fin = (pa[0], emit_bias(pa[0], pa[5]))
                        if h == 2 and pending_fin is not None:
                            emit_finalize(*pending_fin)
                            pending_fin = None
                        pending_attn = (c, h, qwT, kT, wo_t, pzw)
                if pending_attn is not None:
                    pa = pending_attn
                    emit_attn(*pa)
                    if pa[1] == H - 1:
                        pending_fin = (pa[0], emit_bias(pa[0], pa[5]))
                    pending_attn = None
                if pending_fin is not None:
                    emit_finalize(*pending_fin)
                    pending_fin = None
    nc.compile()
    return nc


_CACHED = {}


def _get_nc(rep=1):
    if rep not in _CACHED:
        nc = bacc.Bacc(
            "TRN2", target_bir_lowering=False, debug=False, num_devices=N_CORES
        )
        _CACHED[rep] = build_kernel(nc, rep)
    return _CACHED[rep]


def make_in_maps(x, WQ, WK, WV, Wdot, WO, b):
    """Host-side shard + layout prep. Returns per-core input dicts."""
    bf = ml_dtypes.bfloat16
    x = np.asarray(x, np.float32)
    # fold the bilinear Wdot into WQ: (x @ WQ) @ Wdot == x @ (WQ @ Wdot)
    # x and the projection weights travel as fp16: the 3 extra mantissa bits
    # (vs bf16) matter because x enters the logits quadratically and exp()
    # amplifies correlated rounding; fp16 matmuls run at bf16 speed.
    wq = np.einsum(
        "chdu,chuv->chdv", np.asarray(WQ, np.float32), np.asarray(Wdot, np.float32)
    ).astype(np.float16)
    wk = np.asarray(WK, np.float32).astype(np.float16)
    # swizzle projections to (C, H, 128, KT, U): contiguous per partition
    wq = np.ascontiguousarray(wq.reshape(C, H, KT, 128, U).transpose(0, 1, 3, 2, 4))
    wk = np.ascontiguousarray(wk.reshape(C, H, KT, 128, U).transpose(0, 1, 3, 2, 4))
    # WV -> (D, C*H*UA): per head [v cols 0..95, zero col, v cols 96..99] so
    # the device's ones column (softmax Z accumulator) lands at partition 96.
    wv4 = np.asarray(WV, np.float32).transpose(2, 0, 1, 3)  # (D, C, H, U)
    wv_aug = np.zeros((D, C, H, UA), np.float32)
    wv_aug[..., 0:ZP] = wv4[..., 0:ZP]
    wv_aug[..., ZP + 1 : UA] = wv4[..., ZP:U]
    wv2 = wv_aug.reshape(D, NH * UA).reshape(KT, 128, NH * UA)
    half_w = NH * UA // 2
    wv = np.ascontiguousarray(
        np.stack(
            [wv2[:, :, :half_w].transpose(1, 0, 2), wv2[:, :, half_w:].transpose(1, 0, 2)]
        )
    ).astype(np.float16)  # (2, 128, KT, half_w)
    # WO (C, 800, U) -> per-head blocks (C, H, UA, U) with a zero row at 96
    # matching the v_aug column order (row 96 multiplies the Z row -> 0).
    wo4 = np.asarray(WO, np.float32).reshape(C, H, U, U)
    wo = np.zeros((C, H, UA, U), np.float32)
    wo[:, :, 0:ZP] = wo4[:, :, 0:ZP]
    wo[:, :, ZP + 1 : UA] = wo4[:, :, ZP:U]
    wo = np.ascontiguousarray(wo.transpose(0, 2, 1, 3)).astype(bf)  # (C, UA, H, U)
    bt = np.ascontiguousarray(np.asarray(b, np.float32).T)

    in_maps = []
    for core in range(N_CORES):
        xc = x[core * BC : (core + 1) * BC].reshape(TOK, D)
        xt = np.ascontiguousarray(
            xc.T.reshape(KT, 128, TOK).transpose(1, 0, 2)
        ).astype(np.float16)  # (128, KT, TOK)
        in_maps.append(
            {"xt": xt, "wq": wq, "wk": wk, "wv": wv, "wo": wo, "bt": bt}
        )
    return in_maps


def kernel(x, WQ, WK, WV, Wdot, WO, b):
    nc = _get_nc()
    in_maps = make_in_maps(x, WQ, WK, WV, Wdot, WO, b)
    res = run_bass_kernel_spmd(nc, in_maps, list(range(N_CORES)))
    # gather: per-core out is (C, BC, T, U) -> full (C, B, T, U)
    z = np.concatenate([res.results[core]["out"] for core in range(N_CORES)], axis=1)
    return np.ascontiguousarray(z.astype(np.float32))
